# revision 13
# baseline (speedup 1.0000x reference)
"""Bass/Trainium2 kernel for pre-LN causal attention with RoPE that returns
(out, attn_map, dots).  SPMD over 8 NeuronCores: query rows sharded 512/core.

Per core:
  phase A: LN(x) (E[x^2]-mu^2 variance, gamma folded into W host-side)
           -> xn_T (PE transpose, batched PSUM evictions) -> K/V/Q projections
           (fp32r matmuls), RoPE via shifted-AP reads (3 full DVE passes),
           K/Q transposed to fp32r DRAM scratch, V row-major bf16 scratch.
  phase B: per head h: load q_T_h/k_T_h/v_h; per 128-row q-tile: QK^T (fp32r),
           causal mask (iota+is_gt from per-core row ids, fused into the PSUM
           eviction), dots -> HBM, exp (+row-sum accum) -> normalize -> attn
           -> HBM, PE-transpose attn (4-per-bank batched evict) -> bf16,
           attn_T @ v -> o_T -> DRAM.
  phase C: out = o @ Wo.T [+ bo] + x  -> HBM.
"""
import sys

sys.path.insert(0, "/opt/trn_rl_repo")

import numpy as np

NUM_HEADS = 12
HEAD_DIM = 64
DIM = 768
SEQ = 4096
N_CORES = 8
LC = SEQ // N_CORES          # 512 query rows per core
NT = LC // 128               # 4 query tiles per core
NJC = SEQ // 512             # 8 key chunks
NEG = -np.float32(np.finfo(np.float32).max)

_CACHED = {}


def _build(has_beta, has_bo):
    import concourse.bacc as bacc
    import concourse.mybir as mybir
    import concourse.tile as tile

    F32 = mybir.dt.float32
    F32R = mybir.dt.float32r
    BF16 = mybir.dt.bfloat16
    AF = mybir.ActivationFunctionType
    ALU = mybir.AluOpType

    nc = bacc.Bacc("TRN2", target_bir_lowering=False, debug=False,
                   num_devices=N_CORES)

    # ---- I/O ----
    x_in = nc.dram_tensor("x", [SEQ, DIM], F32, kind="ExternalInput").ap()
    xq_in = nc.dram_tensor("xq", [LC, DIM], F32, kind="ExternalInput").ap()
    cs_in = nc.dram_tensor("cs", [SEQ, 160], F32, kind="ExternalInput").ap()
    csq_in = nc.dram_tensor("csq", [LC, 160], F32, kind="ExternalInput").ap()
    wqt_in = nc.dram_tensor("WqT", [DIM, DIM], F32R, kind="ExternalInput").ap()
    wkt_in = nc.dram_tensor("WkT", [DIM, DIM], F32R, kind="ExternalInput").ap()
    wvt_in = nc.dram_tensor("WvT", [DIM, DIM], F32R, kind="ExternalInput").ap()
    wot_in = nc.dram_tensor("WoT", [DIM, DIM], F32R, kind="ExternalInput").ap()
    if has_beta:
        bet_in = nc.dram_tensor("bet", [128, DIM], F32, kind="ExternalInput").ap()
    if has_bo:
        bo_in = nc.dram_tensor("bo", [128, DIM], F32, kind="ExternalInput").ap()
    gidx_in = nc.dram_tensor("gidx", [128, NT], F32, kind="ExternalInput").ap()
    id_in = nc.dram_tensor("ident", [128, 128], F32, kind="ExternalInput").ap()

    dots_o = nc.dram_tensor("dots", [NUM_HEADS, LC, SEQ], F32,
                            kind="ExternalOutput").ap()
    attn_o = nc.dram_tensor("attn", [NUM_HEADS, LC, SEQ], F32,
                            kind="ExternalOutput").ap()
    out_o = nc.dram_tensor("out", [LC, DIM], F32, kind="ExternalOutput").ap()

    with tile.TileContext(nc) as tc:
        with tc.tile_pool(name="outer", bufs=1) as outer, \
             tc.tile_pool(name="dram", bufs=1, space="DRAM") as dpool:
            ident = outer.tile([128, 128], F32, tag="ident")
            nc.sync.dma_start(out=ident[:, :], in_=id_in[:, :])
            eps = outer.tile([128, 1], F32, tag="eps")
            nc.vector.memset(eps[:, :], 1e-5)
            gidx = outer.tile([128, NT], F32, tag="gidx")
            nc.sync.dma_start(out=gidx[:, :], in_=gidx_in[:, :])
            if has_beta:
                bet = outer.tile([128, DIM], F32, tag="bet")
                nc.sync.dma_start(out=bet[:, :], in_=bet_in[:, :])

            # DRAM scratch, chunked for fine-grained deps
            kts = [dpool.tile([DIM, 512], BF16, tag=f"kts{j}", name=f"kts{j}")
                   for j in range(NJC)]
            vs = [dpool.tile([512, DIM], BF16, tag=f"vs{j}", name=f"vs{j}")
                  for j in range(NJC)]
            qts = dpool.tile([NUM_HEADS, HEAD_DIM, LC], BF16, tag="qts",
                             name="qts")
            ots = dpool.tile([NUM_HEADS, HEAD_DIM, LC], F32R, tag="ots",
                             name="ots")

            # ================= PHASE A =================
            with tc.tile_pool(name="pa_sb", bufs=3) as pa, \
                 tc.tile_pool(name="pa_st", bufs=3) as pst, \
                 tc.tile_pool(name="pa_ps", bufs=2, space="PSUM") as pat, \
                 tc.tile_pool(name="pa_ps2", bufs=2, space="PSUM") as pap:

                def ln_tile(dst, src_rows):
                    """LayerNorm 128 rows (gamma pre-folded into W, beta opt)."""
                    xsb = pa.tile([128, DIM], F32, tag="xsb")
                    nc.sync.dma_start(out=xsb[:, :], in_=src_rows)
                    ssum = pa.tile([128, 1], F32, tag="ssum")
                    sc2 = pa.tile([128, DIM], F32, tag="sc2", bufs=1)
                    nc.scalar.activation(sc2[:, :], xsb[:, :], AF.Copy,
                                         accum_out=ssum[:, :])
                    mu = pa.tile([128, 1], F32, tag="mu")
                    nc.scalar.mul(mu[:, :], ssum[:, :], 1.0 / DIM)
                    sq = pa.tile([128, DIM], F32, tag="sq", bufs=1)
                    s2 = pa.tile([128, 1], F32, tag="s2")
                    nc.scalar.activation(sq[:, :], xsb[:, :], AF.Square,
                                         accum_out=s2[:, :])
                    mu2 = pa.tile([128, 1], F32, tag="mu2")
                    nc.scalar.activation(mu2[:, :], mu[:, :], AF.Square)
                    var = pa.tile([128, 1], F32, tag="var")
                    # var = s2/DIM - mu^2
                    nc.vector.scalar_tensor_tensor(out=var[:, :], in0=s2[:, :],
                                                   scalar=1.0 / DIM,
                                                   in1=mu2[:, :], op0=ALU.mult,
                                                   op1=ALU.subtract)
                    std = pa.tile([128, 1], F32, tag="std")
                    nc.scalar.activation(std[:, :], var[:, :], AF.Sqrt,
                                         bias=eps[:, :])
                    rstd = pa.tile([128, 1], F32, tag="rstd")
                    nc.vector.reciprocal(rstd[:, :], std[:, :])
                    # xn = (x - mu) * rstd   (one DVE pass)
                    nc.vector.tensor_scalar(out=dst, in0=xsb[:, :],
                                            scalar1=mu[:, :], op0=ALU.subtract,
                                            scalar2=rstd[:, :], op1=ALU.mult)
                    if has_beta:
                        nc.vector.tensor_tensor(out=dst, in0=dst, in1=bet[:, :],
                                                op=ALU.add)

                def transpose6(dst_sb, src_sb, r):
                    """6x PE transpose of src [128, 768], single batched evict."""
                    pt6 = pat.tile([128, DIM], F32, tag="pt6")
                    for ct in range(6):
                        nc.tensor.transpose(pt6[:, ct * 128:(ct + 1) * 128],
                                            src_sb[:, ct * 128:(ct + 1) * 128],
                                            ident[:, :])
                    nc.scalar.copy(
                        dst_sb[:, :, r * 128:(r + 1) * 128],
                        pt6[:, :].rearrange("p (c i) -> p c i", c=6))

                def rope(dst, src, cs_sb, final_out=None):
                    """dst = src*cos + rotate_half(src)*sin via shifted APs."""
                    def rep(ap, n32):
                        return ap.rearrange("p (o d) -> p o d", o=1) \
                            .broadcast_to([128, 12, 32 * n32])
                    sv = src.rearrange("p (h d) -> p h d", h=12)
                    rot = pa.tile([128, DIM], F32, tag="rot")
                    rv = rot[:, :].rearrange("p (h d) -> p h d", h=12)
                    # rot[:, :, 0:32]  = src[:, :, 32:64] * (-sin[0:32])
                    nc.vector.tensor_tensor(out=rv[:, :, 0:32],
                                            in0=sv[:, :, 32:64],
                                            in1=rep(cs_sb[:, 128:160], 1),
                                            op=ALU.mult)
                    # rot[:, :, 32:64] = src[:, :, 0:32] * sin[32:64]
                    nc.vector.tensor_tensor(out=rv[:, :, 32:64],
                                            in0=sv[:, :, 0:32],
                                            in1=rep(cs_sb[:, 96:128], 1),
                                            op=ALU.mult)
                    dv = dst.rearrange("p (h d) -> p h d", h=12)
                    nc.vector.tensor_tensor(out=dv, in0=sv,
                                            in1=rep(cs_sb[:, 0:64], 2),
                                            op=ALU.mult)
                    nc.vector.tensor_tensor(
                        out=dst if final_out is None else final_out,
                        in0=dst, in1=rot[:, :], op=ALU.add)

                def project(xnT_sb, w_sb, r, dst):
                    """One 128-row natural-layout projection -> dst [128, 768]."""
                    for half in range(2):
                        pk = pap.tile([128, 384], F32, tag="pk")
                        for ct in range(6):
                            nc.tensor.matmul(
                                pk[:, :],
                                xnT_sb[:, ct, r * 128:(r + 1) * 128],
                                w_sb[:, ct, half * 384:(half + 1) * 384],
                                start=(ct == 0), stop=(ct == 5))
                        nc.scalar.copy(dst[:, half * 384:(half + 1) * 384],
                                       pk[:, :])

                # --- q block (own rows) ---
                with tc.tile_pool(name="pa_q", bufs=1) as paq:
                    wq = paq.tile([128, 6, DIM], F32R, tag="wq")
                    nc.sync.dma_start(out=wq[:, :, :],
                                      in_=wqt_in.rearrange("(t p) a -> p t a",
                                                           p=128))
                    xnTq = paq.tile([128, 6, LC], F32R, tag="xnTq", bufs=1)
                    for r in range(NT):
                        xn = pa.tile([128, DIM], F32, tag="xn")
                        ln_tile(xn[:, :], xq_in[r * 128:(r + 1) * 128, :])
                        transpose6(xnTq[:, :, :], xn[:, :], r)
                    for r in range(NT):
                        qn = pa.tile([128, DIM], F32, tag="qn")
                        project(xnTq, wq, r, qn[:, :])
                        csq_sb = pa.tile([128, 160], F32, tag="csq")
                        nc.sync.dma_start(out=csq_sb[:, :],
                                          in_=csq_in[r * 128:(r + 1) * 128, :])
                        qr = pa.tile([128, DIM], F32, tag="qr")
                        rope(qr[:, :], qn[:, :], csq_sb)
                        qTs = pst.tile([64, 12, 128], BF16, tag="qTs")
                        for hx in range(2):
                            pq6 = pat.tile([64, 6 * 128], F32, tag="pq6", bufs=1)
                            for hh in range(6):
                                h = hx * 6 + hh
                                nc.tensor.transpose(
                                    pq6[:, hh * 128:(hh + 1) * 128],
                                    qr[:, h * 64:(h + 1) * 64], ident[:, :])
                            nc.scalar.copy(
                                qTs[:, hx * 6:(hx + 1) * 6, :],
                                pq6[:, :].rearrange("p (c i) -> p c i", c=6))
                        nc.sync.dma_start(
                            out=qts[:, :, r * 128:(r + 1) * 128]
                            .rearrange("h p i -> p h i"),
                            in_=qTs[:, :, :])

                # --- k/v chunks ---
                with tc.tile_pool(name="pa_kv", bufs=1) as pakv:
                    wk = pakv.tile([128, 6, DIM], F32R, tag="wk")
                    nc.sync.dma_start(out=wk[:, :, :],
                                      in_=wkt_in.rearrange("(t p) a -> p t a",
                                                           p=128))
                    wv = pakv.tile([128, 6, DIM], F32R, tag="wv")
                    nc.sync.dma_start(out=wv[:, :, :],
                                      in_=wvt_in.rearrange("(t p) a -> p t a",
                                                           p=128))
                    for jc in range(NJC):
                        xnT = pst.tile([128, 6, 512], F32R, tag="xnT", bufs=2)
                        for r in range(NT):
                            xn = pa.tile([128, DIM], F32, tag="xn")
                            ln_tile(xn[:, :], x_in[jc * 512 + r * 128:
                                                   jc * 512 + (r + 1) * 128, :])
                            transpose6(xnT[:, :, :], xn[:, :], r)
                        ktT = pst.tile([128, 6, 512], BF16, tag="ktT")
                        vr16 = pst.tile([128, 4, DIM], BF16, tag="vr16")
                        for r in range(NT):
                            cs_sb = pa.tile([128, 160], F32, tag="cs")
                            nc.sync.dma_start(
                                out=cs_sb[:, :],
                                in_=cs_in[jc * 512 + r * 128:
                                          jc * 512 + (r + 1) * 128, :])
                            kn = pa.tile([128, DIM], F32, tag="kn")
                            project(xnT, wk, r, kn[:, :])
                            kr = pa.tile([128, DIM], F32, tag="kr")
                            rope(kr[:, :], kn[:, :], cs_sb)
                            transpose6(ktT[:, :, :], kr[:, :], r)
                            vn = pa.tile([128, DIM], F32, tag="vn")
                            project(xnT, wv, r, vn[:, :])
                            vc = pa.tile([128, DIM], F32, tag="vc")
                            rope(vc[:, :], vn[:, :], cs_sb,
                                 final_out=vr16[:, r, :])
                        nc.sync.dma_start(
                            out=kts[jc][:, :].rearrange("(t p) j -> p t j", p=128),
                            in_=ktT[:, :, :])
                        nc.sync.dma_start(
                            out=vs[jc][:, :].rearrange("(t p) a -> p t a", p=128),
                            in_=vr16[:, :, :])

            # ================= PHASE B =================
            with tc.tile_pool(name="pb_mask", bufs=1) as pbm, \
                 tc.tile_pool(name="pb_sb", bufs=1) as pb1, \
                 tc.tile_pool(name="pb_sm", bufs=2) as pbs, \
                 tc.tile_pool(name="pb_du", bufs=2) as pbd, \
                 tc.tile_pool(name="pb_ps", bufs=3, space="PSUM") as pbq, \
                 tc.tile_pool(name="pb_pst", bufs=3, space="PSUM") as pbt, \
                 tc.tile_pool(name="pb_pso", bufs=2, space="PSUM") as pbo:
                mask = pbm.tile([128, NT, SEQ], F32, tag="mask")
                with tc.tile_pool(name="pb_iota", bufs=1) as pbi:
                    iota = pbi.tile([128, SEQ], F32, tag="iota")
                    nc.gpsimd.iota(iota[:, :], pattern=[[1, SEQ]], base=0,
                                   channel_multiplier=0,
                                   allow_small_or_imprecise_dtypes=True)
                    for t in range(NT):
                        nc.vector.tensor_scalar(out=mask[:, t, :], in0=iota[:, :],
                                                scalar1=gidx[:, t:t + 1],
                                                scalar2=None, op0=ALU.is_gt)
                        nc.vector.tensor_scalar(out=mask[:, t, :],
                                                in0=mask[:, t, :],
                                                scalar1=float(NEG), scalar2=None,
                                                op0=ALU.mult)

                for h in range(NUM_HEADS):
                    ohs = pbs.tile([64, LC], F32R, tag="ohs")
                    qh = pb1.tile([64, LC], BF16, tag="qh")
                    nc.sync.dma_start(out=qh[:, :], in_=qts[h, :, :])
                    kT = pb1.tile([64, NJC, 512], BF16, tag="kT")
                    vh = pb1.tile([128, 32, 64], BF16, tag="vh")
                    for jc in range(NJC):
                        nc.sync.dma_start(out=kT[:, jc, :],
                                          in_=kts[jc][h * 64:(h + 1) * 64, :])
                        nc.sync.dma_start(
                            out=vh[:, jc * 4:(jc + 1) * 4, :],
                            in_=vs[jc][:, h * 64:(h + 1) * 64]
                            .rearrange("(t p) d -> p t d", p=128))
                    for t in range(NT):
                        du = pbd.tile([128, SEQ], F32, tag="du")
                        for jc in range(NJC):
                            pd = pbq.tile([128, 512], F32, tag="pd")
                            nc.tensor.matmul(pd[:, :],
                                             qh[:, t * 128:(t + 1) * 128],
                                             kT[:, jc, :], start=True, stop=True)
                            nc.vector.scalar_tensor_tensor(
                                out=du[:, jc * 512:(jc + 1) * 512], in0=pd[:, :],
                                scalar=0.0,
                                in1=mask[:, t, jc * 512:(jc + 1) * 512],
                                op0=ALU.add, op1=ALU.add)
                            if jc in (1, 3, 5):
                                lo = 1024 * (jc - 1) // 2 * 2
                                lo = {1: 0, 3: 1024, 5: 2048}[jc]
                                nc.sync.dma_start(
                                    out=dots_o[h, t * 128:(t + 1) * 128,
                                               lo:lo + 1024],
                                    in_=du[:, lo:lo + 1024])
                        nc.sync.dma_start(
                            out=dots_o[h, t * 128:(t + 1) * 128, 3072:SEQ],
                            in_=du[:, 3072:SEQ])
                        au = pbd.tile([128, SEQ], F32, tag="au")
                        rs = pbs.tile([128, 1], F32, tag="rs")
                        nc.scalar.activation(au[:, :], du[:, :], AF.Exp,
                                             accum_out=rs[:, :])
                        ri = pbs.tile([128, 1], F32, tag="ri")
                        nc.vector.reciprocal(ri[:, :], rs[:, :])
                        nc.vector.tensor_scalar(out=au[:, :], in0=au[:, :],
                                                scalar1=ri[:, :], scalar2=None,
                                                op0=ALU.mult)
                        nc.sync.dma_start(out=attn_o[h, t * 128:(t + 1) * 128, :],
                                          in_=au[:, :])
                        aT = pbs.tile([128, 32, 128], BF16, tag="aT", bufs=2)
                        for j4 in range(8):
                            pt4 = pbt.tile([128, 512], F32, tag="pt4")
                            for jj in range(4):
                                jt = j4 * 4 + jj
                                nc.tensor.transpose(
                                    pt4[:, jj * 128:(jj + 1) * 128],
                                    au[:, jt * 128:(jt + 1) * 128], ident[:, :])
                            nc.scalar.copy(
                                aT[:, j4 * 4:(j4 + 1) * 4, :],
                                pt4[:, :].rearrange("p (c i) -> p c i", c=4))
                        po = pbo.tile([64, 128], F32, tag="po")
                        for jt in range(32):
                            nc.tensor.matmul(po[:, :], vh[:, jt, :], aT[:, jt, :],
                                             start=(jt == 0), stop=(jt == 31))
                        nc.scalar.copy(ohs[:, t * 128:(t + 1) * 128], po[:, :])
                    nc.sync.dma_start(out=ots[h, :, :], in_=ohs[:, :])

            # ================= PHASE C =================
            with tc.tile_pool(name="pc_sb", bufs=2) as pc, \
                 tc.tile_pool(name="pc_w", bufs=1) as pcw, \
                 tc.tile_pool(name="pc_ps", bufs=2, space="PSUM") as pcp:
                wo = pcw.tile([64, NUM_HEADS, DIM], F32R, tag="wo")
                nc.sync.dma_start(out=wo[:, :, :],
                                  in_=wot_in.rearrange("(h d) e -> d h e", d=64))
                oT64 = pcw.tile([64, NUM_HEADS, LC], F32R, tag="oT64")
                nc.sync.dma_start(out=oT64[:, :, :],
                                  in_=ots[:, :, :].rearrange("h d i -> d h i"))
                if has_bo:
                    bo_sb = pcw.tile([128, DIM], F32, tag="bo")
                    nc.sync.dma_start(out=bo_sb[:, :], in_=bo_in[:, :])
                for t in range(NT):
                    xqb = pc.tile([128, DIM], F32, tag="xqb")
                    nc.sync.dma_start(out=xqb[:, :],
                                      in_=xq_in[t * 128:(t + 1) * 128, :])
                    if has_bo:
                        nc.vector.tensor_tensor(out=xqb[:, :], in0=xqb[:, :],
                                                in1=bo_sb[:, :], op=ALU.add)
                    osb = pc.tile([128, DIM], F32, tag="osb")
                    for half in range(2):
                        pp = pcp.tile([128, 384], F32, tag="pp")
                        for hh in range(NUM_HEADS):
                            nc.tensor.matmul(
                                pp[:, :], oT64[:, hh, t * 128:(t + 1) * 128],
                                wo[:, hh, half * 384:(half + 1) * 384],
                                start=(hh == 0), stop=(hh == NUM_HEADS - 1))
                        nc.vector.scalar_tensor_tensor(
                            out=osb[:, half * 384:(half + 1) * 384], in0=pp[:, :],
                            scalar=0.0, in1=xqb[:, half * 384:(half + 1) * 384],
                            op0=ALU.add, op1=ALU.add)
                    nc.sync.dma_start(out=out_o[t * 128:(t + 1) * 128, :],
                                      in_=osb[:, :])

    nc.compile()
    return nc


def _get_nc(has_beta=False, has_bo=False):
    key = ("nc", has_beta, has_bo)
    if key not in _CACHED:
        _CACHED[key] = _build(has_beta, has_bo)
    return _CACHED[key]


def kernel(x, Wq, Wk, Wv, Wo, bo, gamma, beta, rope):
    from concourse.bass_utils import run_bass_kernel_spmd

    x = np.asarray(x, dtype=np.float32)
    rope = np.asarray(rope, dtype=np.float32)
    x2 = x.reshape(SEQ, DIM)
    ang = rope.reshape(SEQ, HEAD_DIM)
    cosf = np.cos(ang).astype(np.float32)
    sinf = np.sin(ang).astype(np.float32)
    cs = np.concatenate([cosf, sinf, -sinf[:, 0:32]], axis=1).astype(np.float32)
    scale = np.float32(HEAD_DIM ** -0.5)
    gamma = np.asarray(gamma, np.float32)
    beta = np.asarray(beta, np.float32)
    bo = np.asarray(bo, np.float32)
    has_beta = bool(np.any(beta != 0))
    has_bo = bool(np.any(bo != 0))
    # fold gamma into the projection weights (W @ diag(gamma) transposed)
    WqT = np.ascontiguousarray((np.asarray(Wq) * scale).T.astype(np.float32)
                               * gamma[:, None])
    WkT = np.ascontiguousarray(np.asarray(Wk).T.astype(np.float32)
                               * gamma[:, None])
    WvT = np.ascontiguousarray(np.asarray(Wv).T.astype(np.float32)
                               * gamma[:, None])
    WoT = np.ascontiguousarray(np.asarray(Wo).T.astype(np.float32))
    ident = np.eye(128, dtype=np.float32)

    in_maps = []
    for c in range(N_CORES):
        rows = slice(c * LC, (c + 1) * LC)
        gidx = (np.arange(c * LC, (c + 1) * LC, dtype=np.float32)
                .reshape(NT, 128).T.copy())
        im = {
            "x": x2, "xq": np.ascontiguousarray(x2[rows]),
            "cs": cs, "csq": np.ascontiguousarray(cs[rows]),
            "WqT": WqT, "WkT": WkT, "WvT": WvT, "WoT": WoT,
            "gidx": gidx, "ident": ident,
        }
        if has_beta:
            im["bet"] = np.broadcast_to(beta / np.where(gamma == 0, 1, gamma), (128, DIM)).astype(np.float32).copy()
        if has_bo:
            im["bo"] = np.broadcast_to(bo, (128, DIM)).copy()
        in_maps.append(im)

    nc = _get_nc(has_beta, has_bo)
    res = run_bass_kernel_spmd(nc, in_maps, core_ids=list(range(N_CORES)))

    out = np.empty((1, SEQ, DIM), np.float32)
    attn_map = np.empty((1, NUM_HEADS, SEQ, SEQ), np.float32)
    dots = np.empty((1, NUM_HEADS, SEQ, SEQ), np.float32)
    for c in range(N_CORES):
        r = res.results[c]
        rows = slice(c * LC, (c + 1) * LC)
        out[0, rows] = r["out"]
        attn_map[0, :, rows] = r["attn"]
        dots[0, :, rows] = r["dots"]
    return out, attn_map, dots


# revision 15
# speedup vs baseline: 1.0469x; 1.0469x over previous
"""Bass/Trainium2 kernel for pre-LN causal attention with RoPE that returns
(out, attn_map, dots).  SPMD over 8 NeuronCores: query rows sharded 512/core.

Per core:
  phase A: LN(x) (E[x^2]-mu^2 variance, gamma folded into W host-side)
           -> xn_T (PE transpose, batched PSUM evictions) -> K/V/Q projections
           (fp32r matmuls), RoPE via shifted-AP reads (3 full DVE passes),
           K/Q transposed to fp32r DRAM scratch, V row-major bf16 scratch.
  phase B: per head h: load q_T_h/k_T_h/v_h; per 128-row q-tile: QK^T (fp32r),
           causal mask (iota+is_gt from per-core row ids, fused into the PSUM
           eviction), dots -> HBM, exp (+row-sum accum) -> normalize -> attn
           -> HBM, PE-transpose attn (4-per-bank batched evict) -> bf16,
           attn_T @ v -> o_T -> DRAM.
  phase C: out = o @ Wo.T [+ bo] + x  -> HBM.
"""
import sys

sys.path.insert(0, "/opt/trn_rl_repo")

import numpy as np

NUM_HEADS = 12
HEAD_DIM = 64
DIM = 768
SEQ = 4096
N_CORES = 8
LC = SEQ // N_CORES          # 512 query rows per core
NT = LC // 128               # 4 query tiles per core
NJC = SEQ // 512             # 8 key chunks
NEG = -np.float32(np.finfo(np.float32).max)

_CACHED = {}


def _build(has_beta, has_bo):
    import concourse.bacc as bacc
    import concourse.mybir as mybir
    import concourse.tile as tile

    F32 = mybir.dt.float32
    F32R = mybir.dt.float32r
    BF16 = mybir.dt.bfloat16
    AF = mybir.ActivationFunctionType
    ALU = mybir.AluOpType

    nc = bacc.Bacc("TRN2", target_bir_lowering=False, debug=False,
                   num_devices=N_CORES)

    # ---- I/O ----
    x_in = nc.dram_tensor("x", [SEQ, DIM], F32, kind="ExternalInput").ap()
    xq_in = nc.dram_tensor("xq", [LC, DIM], F32, kind="ExternalInput").ap()
    cs_in = nc.dram_tensor("cs", [SEQ, 160], F32, kind="ExternalInput").ap()
    csq_in = nc.dram_tensor("csq", [LC, 160], F32, kind="ExternalInput").ap()
    wqt_in = nc.dram_tensor("WqT", [DIM, DIM], F32R, kind="ExternalInput").ap()
    wkt_in = nc.dram_tensor("WkT", [DIM, DIM], F32R, kind="ExternalInput").ap()
    wvt_in = nc.dram_tensor("WvT", [DIM, DIM], F32R, kind="ExternalInput").ap()
    wot_in = nc.dram_tensor("WoT", [DIM, DIM], F32R, kind="ExternalInput").ap()
    if has_beta:
        bet_in = nc.dram_tensor("bet", [128, DIM], F32, kind="ExternalInput").ap()
    if has_bo:
        bo_in = nc.dram_tensor("bo", [128, DIM], F32, kind="ExternalInput").ap()
    gidx_in = nc.dram_tensor("gidx", [128, NT], F32, kind="ExternalInput").ap()
    id_in = nc.dram_tensor("ident", [128, 128], F32, kind="ExternalInput").ap()
    id16_in = nc.dram_tensor("ident16", [128, 128], BF16, kind="ExternalInput").ap()
    cs16_in = nc.dram_tensor("cs16", [SEQ, 160], BF16, kind="ExternalInput").ap()

    dots_o = nc.dram_tensor("dots", [NUM_HEADS, LC, SEQ], F32,
                            kind="ExternalOutput").ap()
    attn_o = nc.dram_tensor("attn", [NUM_HEADS, LC, SEQ], F32,
                            kind="ExternalOutput").ap()
    out_o = nc.dram_tensor("out", [LC, DIM], F32, kind="ExternalOutput").ap()

    with tile.TileContext(nc) as tc:
        with tc.tile_pool(name="outer", bufs=1) as outer, \
             tc.tile_pool(name="dram", bufs=1, space="DRAM") as dpool:
            ident = outer.tile([128, 128], F32, tag="ident")
            nc.sync.dma_start(out=ident[:, :], in_=id_in[:, :])
            ident16 = outer.tile([128, 128], BF16, tag="ident16")
            nc.sync.dma_start(out=ident16[:, :], in_=id16_in[:, :])
            eps = outer.tile([128, 1], F32, tag="eps")
            nc.vector.memset(eps[:, :], 1e-5)
            gidx = outer.tile([128, NT], F32, tag="gidx")
            nc.sync.dma_start(out=gidx[:, :], in_=gidx_in[:, :])
            if has_beta:
                bet = outer.tile([128, DIM], F32, tag="bet")
                nc.sync.dma_start(out=bet[:, :], in_=bet_in[:, :])

            # DRAM scratch, chunked for fine-grained deps
            kts = [dpool.tile([DIM, 512], BF16, tag=f"kts{j}", name=f"kts{j}")
                   for j in range(NJC)]
            vs = [dpool.tile([512, DIM], BF16, tag=f"vs{j}", name=f"vs{j}")
                  for j in range(NJC)]
            qts = dpool.tile([NUM_HEADS, HEAD_DIM, LC], BF16, tag="qts",
                             name="qts")
            ots = dpool.tile([NUM_HEADS, HEAD_DIM, LC], F32R, tag="ots",
                             name="ots")

            # ================= PHASE A =================
            with tc.tile_pool(name="pa_sb", bufs=3) as pa, \
                 tc.tile_pool(name="pa_st", bufs=3) as pst, \
                 tc.tile_pool(name="pa_ps", bufs=2, space="PSUM") as pat, \
                 tc.tile_pool(name="pa_ps2", bufs=2, space="PSUM") as pap:

                def ln_tile(dst, src_rows):
                    """LayerNorm 128 rows (gamma pre-folded into W, beta opt)."""
                    xsb = pa.tile([128, DIM], F32, tag="xsb")
                    nc.sync.dma_start(out=xsb[:, :], in_=src_rows)
                    ssum = pa.tile([128, 1], F32, tag="ssum")
                    sc2 = pa.tile([128, DIM], F32, tag="sc2", bufs=1)
                    nc.scalar.activation(sc2[:, :], xsb[:, :], AF.Copy,
                                         accum_out=ssum[:, :])
                    mu = pa.tile([128, 1], F32, tag="mu")
                    nc.scalar.mul(mu[:, :], ssum[:, :], 1.0 / DIM)
                    sq = pa.tile([128, DIM], F32, tag="sq", bufs=1)
                    s2 = pa.tile([128, 1], F32, tag="s2")
                    nc.scalar.activation(sq[:, :], xsb[:, :], AF.Square,
                                         accum_out=s2[:, :])
                    mu2 = pa.tile([128, 1], F32, tag="mu2")
                    nc.scalar.activation(mu2[:, :], mu[:, :], AF.Square)
                    var = pa.tile([128, 1], F32, tag="var")
                    # var = s2/DIM - mu^2
                    nc.vector.scalar_tensor_tensor(out=var[:, :], in0=s2[:, :],
                                                   scalar=1.0 / DIM,
                                                   in1=mu2[:, :], op0=ALU.mult,
                                                   op1=ALU.subtract)
                    std = pa.tile([128, 1], F32, tag="std")
                    nc.scalar.activation(std[:, :], var[:, :], AF.Sqrt,
                                         bias=eps[:, :])
                    rstd = pa.tile([128, 1], F32, tag="rstd")
                    nc.vector.reciprocal(rstd[:, :], std[:, :])
                    # xn = (x - mu) * rstd   (one DVE pass)
                    nc.vector.tensor_scalar(out=dst, in0=xsb[:, :],
                                            scalar1=mu[:, :], op0=ALU.subtract,
                                            scalar2=rstd[:, :], op1=ALU.mult)
                    if has_beta:
                        nc.vector.tensor_tensor(out=dst, in0=dst, in1=bet[:, :],
                                                op=ALU.add)

                def transpose6(dst_sb, src_sb, r, bf=False):
                    """6x PE transpose of src [128, 768], single batched evict."""
                    pt6 = pat.tile([128, DIM], BF16 if bf else F32,
                                   tag="pt6b" if bf else "pt6",
                                   bufs=2 if bf else 1)
                    for ct in range(6):
                        nc.tensor.transpose(pt6[:, ct * 128:(ct + 1) * 128],
                                            src_sb[:, ct * 128:(ct + 1) * 128],
                                            ident16[:, :] if bf else ident[:, :])
                    nc.scalar.copy(
                        dst_sb[:, :, r * 128:(r + 1) * 128],
                        pt6[:, :].rearrange("p (c i) -> p c i", c=6))

                def rope(dst, src, cs_sb, final_out=None):
                    """dst = src*cos + rotate_half(src)*sin via shifted APs."""
                    def rep(ap, n32):
                        return ap.rearrange("p (o d) -> p o d", o=1) \
                            .broadcast_to([128, 12, 32 * n32])
                    sv = src.rearrange("p (h d) -> p h d", h=12)
                    rot = pa.tile([128, DIM], F32, tag="rot")
                    rv = rot[:, :].rearrange("p (h d) -> p h d", h=12)
                    # rot[:, :, 0:32]  = src[:, :, 32:64] * (-sin[0:32])
                    nc.vector.tensor_tensor(out=rv[:, :, 0:32],
                                            in0=sv[:, :, 32:64],
                                            in1=rep(cs_sb[:, 128:160], 1),
                                            op=ALU.mult)
                    # rot[:, :, 32:64] = src[:, :, 0:32] * sin[32:64]
                    nc.vector.tensor_tensor(out=rv[:, :, 32:64],
                                            in0=sv[:, :, 0:32],
                                            in1=rep(cs_sb[:, 96:128], 1),
                                            op=ALU.mult)
                    dv = dst.rearrange("p (h d) -> p h d", h=12)
                    nc.vector.tensor_tensor(out=dv, in0=sv,
                                            in1=rep(cs_sb[:, 0:64], 2),
                                            op=ALU.mult)
                    nc.vector.tensor_tensor(
                        out=dst if final_out is None else final_out,
                        in0=dst, in1=rot[:, :], op=ALU.add)

                def rope16(dst, src, cs_sb, final_out=None):
                    """bf16 rope via shifted APs (DVE 4x mode)."""
                    def rep(ap, n32):
                        return ap.rearrange("p (o d) -> p o d", o=1) \
                            .broadcast_to([128, 12, 32 * n32])
                    sv = src.rearrange("p (h d) -> p h d", h=12)
                    rot = pa.tile([128, DIM], BF16, tag="rot16")
                    rv = rot[:, :].rearrange("p (h d) -> p h d", h=12)
                    nc.vector.tensor_tensor(out=rv[:, :, 0:32],
                                            in0=sv[:, :, 32:64],
                                            in1=rep(cs_sb[:, 128:160], 1),
                                            op=ALU.mult)
                    nc.vector.tensor_tensor(out=rv[:, :, 32:64],
                                            in0=sv[:, :, 0:32],
                                            in1=rep(cs_sb[:, 96:128], 1),
                                            op=ALU.mult)
                    dv = dst.rearrange("p (h d) -> p h d", h=12)
                    nc.vector.tensor_tensor(out=dv, in0=sv,
                                            in1=rep(cs_sb[:, 0:64], 2),
                                            op=ALU.mult)
                    nc.vector.tensor_tensor(
                        out=dst if final_out is None else final_out,
                        in0=dst, in1=rot[:, :], op=ALU.add)

                def project(xnT_sb, w_sb, r, dst):
                    """One 128-row natural-layout projection -> dst [128, 768]."""
                    for half in range(2):
                        pk = pap.tile([128, 384], F32, tag="pk")
                        for ct in range(6):
                            nc.tensor.matmul(
                                pk[:, :],
                                xnT_sb[:, ct, r * 128:(r + 1) * 128],
                                w_sb[:, ct, half * 384:(half + 1) * 384],
                                start=(ct == 0), stop=(ct == 5))
                        nc.scalar.copy(dst[:, half * 384:(half + 1) * 384],
                                       pk[:, :])

                # --- q block (own rows) ---
                with tc.tile_pool(name="pa_q", bufs=1) as paq:
                    wq = paq.tile([128, 6, DIM], F32R, tag="wq")
                    nc.sync.dma_start(out=wq[:, :, :],
                                      in_=wqt_in.rearrange("(t p) a -> p t a",
                                                           p=128))
                    xnTq = paq.tile([128, 6, LC], F32R, tag="xnTq", bufs=1)
                    for r in range(NT):
                        xn = pa.tile([128, DIM], F32, tag="xn")
                        ln_tile(xn[:, :], xq_in[r * 128:(r + 1) * 128, :])
                        transpose6(xnTq[:, :, :], xn[:, :], r)
                    for r in range(NT):
                        qn = pa.tile([128, DIM], F32, tag="qn")
                        project(xnTq, wq, r, qn[:, :])
                        csq_sb = pa.tile([128, 160], F32, tag="csq")
                        nc.sync.dma_start(out=csq_sb[:, :],
                                          in_=csq_in[r * 128:(r + 1) * 128, :])
                        qr = pa.tile([128, DIM], F32, tag="qr")
                        rope(qr[:, :], qn[:, :], csq_sb)
                        qTs = pst.tile([64, 12, 128], BF16, tag="qTs")
                        for hx in range(2):
                            pq6 = pat.tile([64, 6 * 128], F32, tag="pq6", bufs=1)
                            for hh in range(6):
                                h = hx * 6 + hh
                                nc.tensor.transpose(
                                    pq6[:, hh * 128:(hh + 1) * 128],
                                    qr[:, h * 64:(h + 1) * 64], ident[:, :])
                            nc.scalar.copy(
                                qTs[:, hx * 6:(hx + 1) * 6, :],
                                pq6[:, :].rearrange("p (c i) -> p c i", c=6))
                        nc.sync.dma_start(
                            out=qts[:, :, r * 128:(r + 1) * 128]
                            .rearrange("h p i -> p h i"),
                            in_=qTs[:, :, :])

                # --- k/v chunks ---
                with tc.tile_pool(name="pa_kv", bufs=1) as pakv:
                    wk = pakv.tile([128, 6, DIM], F32R, tag="wk")
                    nc.sync.dma_start(out=wk[:, :, :],
                                      in_=wkt_in.rearrange("(t p) a -> p t a",
                                                           p=128))
                    wv = pakv.tile([128, 6, DIM], F32R, tag="wv")
                    nc.sync.dma_start(out=wv[:, :, :],
                                      in_=wvt_in.rearrange("(t p) a -> p t a",
                                                           p=128))
                    for jc in range(NJC):
                        xnT = pst.tile([128, 6, 512], F32R, tag="xnT", bufs=2)
                        for r in range(NT):
                            xn = pa.tile([128, DIM], F32, tag="xn")
                            ln_tile(xn[:, :], x_in[jc * 512 + r * 128:
                                                   jc * 512 + (r + 1) * 128, :])
                            transpose6(xnT[:, :, :], xn[:, :], r)
                        ktT = pst.tile([128, 6, 512], BF16, tag="ktT")
                        vr16 = pst.tile([128, 4, DIM], BF16, tag="vr16")
                        for r in range(NT):
                            cs_sb = pa.tile([128, 160], BF16, tag="cs")
                            nc.sync.dma_start(
                                out=cs_sb[:, :],
                                in_=cs16_in[jc * 512 + r * 128:
                                            jc * 512 + (r + 1) * 128, :])
                            kn = pa.tile([128, DIM], BF16, tag="kn")
                            project(xnT, wk, r, kn[:, :])
                            kr = pa.tile([128, DIM], BF16, tag="kr")
                            rope16(kr[:, :], kn[:, :], cs_sb)
                            transpose6(ktT[:, :, :], kr[:, :], r, bf=True)
                            vn = pa.tile([128, DIM], BF16, tag="vn")
                            project(xnT, wv, r, vn[:, :])
                            vc = pa.tile([128, DIM], BF16, tag="vc")
                            rope16(vc[:, :], vn[:, :], cs_sb,
                                   final_out=vr16[:, r, :])
                        nc.sync.dma_start(
                            out=kts[jc][:, :].rearrange("(t p) j -> p t j", p=128),
                            in_=ktT[:, :, :])
                        nc.sync.dma_start(
                            out=vs[jc][:, :].rearrange("(t p) a -> p t a", p=128),
                            in_=vr16[:, :, :])

            # ================= PHASE B =================
            with tc.tile_pool(name="pb_mask", bufs=1) as pbm, \
                 tc.tile_pool(name="pb_sb", bufs=1) as pb1, \
                 tc.tile_pool(name="pb_sm", bufs=2) as pbs, \
                 tc.tile_pool(name="pb_du", bufs=2) as pbd, \
                 tc.tile_pool(name="pb_ps", bufs=3, space="PSUM") as pbq, \
                 tc.tile_pool(name="pb_pst", bufs=3, space="PSUM") as pbt, \
                 tc.tile_pool(name="pb_pso", bufs=2, space="PSUM") as pbo:
                mask = pbm.tile([128, NT, SEQ], F32, tag="mask")
                with tc.tile_pool(name="pb_iota", bufs=1) as pbi:
                    iota = pbi.tile([128, SEQ], F32, tag="iota")
                    nc.gpsimd.iota(iota[:, :], pattern=[[1, SEQ]], base=0,
                                   channel_multiplier=0,
                                   allow_small_or_imprecise_dtypes=True)
                    for t in range(NT):
                        nc.vector.tensor_scalar(out=mask[:, t, :], in0=iota[:, :],
                                                scalar1=gidx[:, t:t + 1],
                                                scalar2=None, op0=ALU.is_gt)
                        nc.vector.tensor_scalar(out=mask[:, t, :],
                                                in0=mask[:, t, :],
                                                scalar1=float(NEG), scalar2=None,
                                                op0=ALU.mult)

                for h in range(NUM_HEADS):
                    ohs = pbs.tile([64, LC], F32R, tag="ohs")
                    qh = pb1.tile([64, LC], BF16, tag="qh", bufs=2)
                    nc.sync.dma_start(out=qh[:, :], in_=qts[h, :, :])
                    kT = pb1.tile([64, NJC, 512], BF16, tag="kT", bufs=2)
                    vh = pb1.tile([128, 32, 64], BF16, tag="vh", bufs=2)
                    for jc in range(NJC):
                        nc.sync.dma_start(out=kT[:, jc, :],
                                          in_=kts[jc][h * 64:(h + 1) * 64, :])
                        nc.sync.dma_start(
                            out=vh[:, jc * 4:(jc + 1) * 4, :],
                            in_=vs[jc][:, h * 64:(h + 1) * 64]
                            .rearrange("(t p) d -> p t d", p=128))
                    for t in range(NT):
                        du = pbd.tile([128, SEQ], F32, tag="du", bufs=3)
                        for jc in range(NJC):
                            pd = pbq.tile([128, 512], F32, tag="pd")
                            nc.tensor.matmul(pd[:, :],
                                             qh[:, t * 128:(t + 1) * 128],
                                             kT[:, jc, :], start=True, stop=True)
                            nc.vector.scalar_tensor_tensor(
                                out=du[:, jc * 512:(jc + 1) * 512], in0=pd[:, :],
                                scalar=0.0,
                                in1=mask[:, t, jc * 512:(jc + 1) * 512],
                                op0=ALU.add, op1=ALU.add)
                            if jc in (1, 3, 5):
                                lo = 1024 * (jc - 1) // 2 * 2
                                lo = {1: 0, 3: 1024, 5: 2048}[jc]
                                nc.sync.dma_start(
                                    out=dots_o[h, t * 128:(t + 1) * 128,
                                               lo:lo + 1024],
                                    in_=du[:, lo:lo + 1024])
                        nc.sync.dma_start(
                            out=dots_o[h, t * 128:(t + 1) * 128, 3072:SEQ],
                            in_=du[:, 3072:SEQ])
                        au = pbd.tile([128, SEQ], F32, tag="au")
                        rs = pbs.tile([128, 1], F32, tag="rs")
                        nc.scalar.activation(au[:, :], du[:, :], AF.Exp,
                                             accum_out=rs[:, :])
                        ri = pbs.tile([128, 1], F32, tag="ri")
                        nc.vector.reciprocal(ri[:, :], rs[:, :])
                        nc.vector.tensor_scalar(out=au[:, :], in0=au[:, :],
                                                scalar1=ri[:, :], scalar2=None,
                                                op0=ALU.mult)
                        nc.sync.dma_start(out=attn_o[h, t * 128:(t + 1) * 128, :],
                                          in_=au[:, :])
                        aT = pbs.tile([128, 32, 128], BF16, tag="aT", bufs=2)
                        for j4 in range(8):
                            pt4 = pbt.tile([128, 512], F32, tag="pt4")
                            for jj in range(4):
                                jt = j4 * 4 + jj
                                nc.tensor.transpose(
                                    pt4[:, jj * 128:(jj + 1) * 128],
                                    au[:, jt * 128:(jt + 1) * 128], ident[:, :])
                            nc.scalar.copy(
                                aT[:, j4 * 4:(j4 + 1) * 4, :],
                                pt4[:, :].rearrange("p (c i) -> p c i", c=4))
                        po = pbo.tile([64, 128], F32, tag="po")
                        for jt in range(32):
                            nc.tensor.matmul(po[:, :], vh[:, jt, :], aT[:, jt, :],
                                             start=(jt == 0), stop=(jt == 31))
                        nc.scalar.copy(ohs[:, t * 128:(t + 1) * 128], po[:, :])
                    nc.sync.dma_start(out=ots[h, :, :], in_=ohs[:, :])

            # ================= PHASE C =================
            with tc.tile_pool(name="pc_sb", bufs=2) as pc, \
                 tc.tile_pool(name="pc_w", bufs=1) as pcw, \
                 tc.tile_pool(name="pc_ps", bufs=2, space="PSUM") as pcp:
                wo = pcw.tile([64, NUM_HEADS, DIM], F32R, tag="wo")
                nc.sync.dma_start(out=wo[:, :, :],
                                  in_=wot_in.rearrange("(h d) e -> d h e", d=64))
                oT64 = pcw.tile([64, NUM_HEADS, LC], F32R, tag="oT64")
                nc.sync.dma_start(out=oT64[:, :, :],
                                  in_=ots[:, :, :].rearrange("h d i -> d h i"))
                if has_bo:
                    bo_sb = pcw.tile([128, DIM], F32, tag="bo")
                    nc.sync.dma_start(out=bo_sb[:, :], in_=bo_in[:, :])
                for t in range(NT):
                    xqb = pc.tile([128, DIM], F32, tag="xqb")
                    nc.sync.dma_start(out=xqb[:, :],
                                      in_=xq_in[t * 128:(t + 1) * 128, :])
                    if has_bo:
                        nc.vector.tensor_tensor(out=xqb[:, :], in0=xqb[:, :],
                                                in1=bo_sb[:, :], op=ALU.add)
                    osb = pc.tile([128, DIM], F32, tag="osb")
                    for half in range(2):
                        pp = pcp.tile([128, 384], F32, tag="pp")
                        for hh in range(NUM_HEADS):
                            nc.tensor.matmul(
                                pp[:, :], oT64[:, hh, t * 128:(t + 1) * 128],
                                wo[:, hh, half * 384:(half + 1) * 384],
                                start=(hh == 0), stop=(hh == NUM_HEADS - 1))
                        nc.vector.scalar_tensor_tensor(
                            out=osb[:, half * 384:(half + 1) * 384], in0=pp[:, :],
                            scalar=0.0, in1=xqb[:, half * 384:(half + 1) * 384],
                            op0=ALU.add, op1=ALU.add)
                    nc.sync.dma_start(out=out_o[t * 128:(t + 1) * 128, :],
                                      in_=osb[:, :])

    nc.compile()
    return nc


def _get_nc(has_beta=False, has_bo=False):
    key = ("nc", has_beta, has_bo)
    if key not in _CACHED:
        _CACHED[key] = _build(has_beta, has_bo)
    return _CACHED[key]


def kernel(x, Wq, Wk, Wv, Wo, bo, gamma, beta, rope):
    from concourse.bass_utils import run_bass_kernel_spmd

    x = np.asarray(x, dtype=np.float32)
    rope = np.asarray(rope, dtype=np.float32)
    x2 = x.reshape(SEQ, DIM)
    ang = rope.reshape(SEQ, HEAD_DIM)
    cosf = np.cos(ang).astype(np.float32)
    sinf = np.sin(ang).astype(np.float32)
    cs = np.concatenate([cosf, sinf, -sinf[:, 0:32]], axis=1).astype(np.float32)
    scale = np.float32(HEAD_DIM ** -0.5)
    gamma = np.asarray(gamma, np.float32)
    beta = np.asarray(beta, np.float32)
    bo = np.asarray(bo, np.float32)
    has_beta = bool(np.any(beta != 0))
    has_bo = bool(np.any(bo != 0))
    # fold gamma into the projection weights (W @ diag(gamma) transposed)
    WqT = np.ascontiguousarray((np.asarray(Wq) * scale).T.astype(np.float32)
                               * gamma[:, None])
    WkT = np.ascontiguousarray(np.asarray(Wk).T.astype(np.float32)
                               * gamma[:, None])
    WvT = np.ascontiguousarray(np.asarray(Wv).T.astype(np.float32)
                               * gamma[:, None])
    WoT = np.ascontiguousarray(np.asarray(Wo).T.astype(np.float32))
    ident = np.eye(128, dtype=np.float32)
    import ml_dtypes
    ident16 = np.eye(128, dtype=ml_dtypes.bfloat16)
    cs16 = cs.astype(ml_dtypes.bfloat16)

    in_maps = []
    for c in range(N_CORES):
        rows = slice(c * LC, (c + 1) * LC)
        gidx = (np.arange(c * LC, (c + 1) * LC, dtype=np.float32)
                .reshape(NT, 128).T.copy())
        im = {
            "x": x2, "xq": np.ascontiguousarray(x2[rows]),
            "cs": cs, "csq": np.ascontiguousarray(cs[rows]),
            "WqT": WqT, "WkT": WkT, "WvT": WvT, "WoT": WoT,
            "gidx": gidx, "ident": ident, "ident16": ident16, "cs16": cs16,
        }
        if has_beta:
            im["bet"] = np.broadcast_to(beta / np.where(gamma == 0, 1, gamma), (128, DIM)).astype(np.float32).copy()
        if has_bo:
            im["bo"] = np.broadcast_to(bo, (128, DIM)).copy()
        in_maps.append(im)

    nc = _get_nc(has_beta, has_bo)
    res = run_bass_kernel_spmd(nc, in_maps, core_ids=list(range(N_CORES)))

    out = np.empty((1, SEQ, DIM), np.float32)
    attn_map = np.empty((1, NUM_HEADS, SEQ, SEQ), np.float32)
    dots = np.empty((1, NUM_HEADS, SEQ, SEQ), np.float32)
    for c in range(N_CORES):
        r = res.results[c]
        rows = slice(c * LC, (c + 1) * LC)
        out[0, rows] = r["out"]
        attn_map[0, :, rows] = r["attn"]
        dots[0, :, rows] = r["dots"]
    return out, attn_map, dots


# revision 16
# speedup vs baseline: 1.0596x; 1.0121x over previous
"""Bass/Trainium2 kernel for pre-LN causal attention with RoPE that returns
(out, attn_map, dots).  SPMD over 8 NeuronCores: query rows sharded 512/core.

Per core:
  phase A: LN(x) (E[x^2]-mu^2 variance, gamma folded into W host-side)
           -> xn_T (PE transpose, batched PSUM evictions) -> K/V/Q projections
           (fp32r matmuls), RoPE via shifted-AP reads (3 full DVE passes),
           K/Q transposed to fp32r DRAM scratch, V row-major bf16 scratch.
  phase B: per head h: load q_T_h/k_T_h/v_h; per 128-row q-tile: QK^T (fp32r),
           causal mask (iota+is_gt from per-core row ids, fused into the PSUM
           eviction), dots -> HBM, exp (+row-sum accum) -> normalize -> attn
           -> HBM, PE-transpose attn (4-per-bank batched evict) -> bf16,
           attn_T @ v -> o_T -> DRAM.
  phase C: out = o @ Wo.T [+ bo] + x  -> HBM.
"""
import sys

sys.path.insert(0, "/opt/trn_rl_repo")

import numpy as np

NUM_HEADS = 12
HEAD_DIM = 64
DIM = 768
SEQ = 4096
N_CORES = 8
LC = SEQ // N_CORES          # 512 query rows per core
NT = LC // 128               # 4 query tiles per core
NJC = SEQ // 512             # 8 key chunks
NEG = -np.float32(np.finfo(np.float32).max)

_CACHED = {}


def _build(has_beta, has_bo):
    import concourse.bacc as bacc
    import concourse.mybir as mybir
    import concourse.tile as tile

    F32 = mybir.dt.float32
    F32R = mybir.dt.float32r
    BF16 = mybir.dt.bfloat16
    AF = mybir.ActivationFunctionType
    ALU = mybir.AluOpType

    nc = bacc.Bacc("TRN2", target_bir_lowering=False, debug=False,
                   num_devices=N_CORES)

    # ---- I/O ----
    x_in = nc.dram_tensor("x", [SEQ, DIM], F32, kind="ExternalInput").ap()
    xq_in = nc.dram_tensor("xq", [LC, DIM], F32, kind="ExternalInput").ap()
    cs_in = nc.dram_tensor("cs", [SEQ, 160], F32, kind="ExternalInput").ap()
    csq_in = nc.dram_tensor("csq", [LC, 160], F32, kind="ExternalInput").ap()
    wqt_in = nc.dram_tensor("WqT", [DIM, DIM], F32R, kind="ExternalInput").ap()
    wkt_in = nc.dram_tensor("WkT", [DIM, DIM], F32R, kind="ExternalInput").ap()
    wvt_in = nc.dram_tensor("WvT", [DIM, DIM], F32R, kind="ExternalInput").ap()
    wot_in = nc.dram_tensor("WoT", [DIM, DIM], F32R, kind="ExternalInput").ap()
    if has_beta:
        bet_in = nc.dram_tensor("bet", [128, DIM], F32, kind="ExternalInput").ap()
    if has_bo:
        bo_in = nc.dram_tensor("bo", [128, DIM], F32, kind="ExternalInput").ap()
    gidx_in = nc.dram_tensor("gidx", [128, NT], F32, kind="ExternalInput").ap()
    id_in = nc.dram_tensor("ident", [128, 128], F32, kind="ExternalInput").ap()
    id16_in = nc.dram_tensor("ident16", [128, 128], BF16, kind="ExternalInput").ap()
    cs16_in = nc.dram_tensor("cs16", [SEQ, 160], BF16, kind="ExternalInput").ap()

    dots_o = nc.dram_tensor("dots", [NUM_HEADS, LC, SEQ], F32,
                            kind="ExternalOutput").ap()
    attn_o = nc.dram_tensor("attn", [NUM_HEADS, LC, SEQ], F32,
                            kind="ExternalOutput").ap()
    out_o = nc.dram_tensor("out", [LC, DIM], F32, kind="ExternalOutput").ap()

    with tile.TileContext(nc) as tc:
        with tc.tile_pool(name="outer", bufs=1) as outer, \
             tc.tile_pool(name="dram", bufs=1, space="DRAM") as dpool:
            ident = outer.tile([128, 128], F32, tag="ident")
            nc.sync.dma_start(out=ident[:, :], in_=id_in[:, :])
            ident16 = outer.tile([128, 128], BF16, tag="ident16")
            nc.sync.dma_start(out=ident16[:, :], in_=id16_in[:, :])
            eps = outer.tile([128, 1], F32, tag="eps")
            nc.vector.memset(eps[:, :], 1e-5)
            gidx = outer.tile([128, NT], F32, tag="gidx")
            nc.sync.dma_start(out=gidx[:, :], in_=gidx_in[:, :])
            if has_beta:
                bet = outer.tile([128, DIM], F32, tag="bet")
                nc.sync.dma_start(out=bet[:, :], in_=bet_in[:, :])

            # DRAM scratch, chunked for fine-grained deps
            kts = [dpool.tile([DIM, 512], BF16, tag=f"kts{j}", name=f"kts{j}")
                   for j in range(NJC)]
            vs = [dpool.tile([512, DIM], BF16, tag=f"vs{j}", name=f"vs{j}")
                  for j in range(NJC)]
            qts = dpool.tile([NUM_HEADS, HEAD_DIM, LC], BF16, tag="qts",
                             name="qts")
            ots = dpool.tile([NUM_HEADS, HEAD_DIM, LC], F32R, tag="ots",
                             name="ots")

            # ================= PHASE A =================
            with tc.tile_pool(name="pa_sb", bufs=3) as pa, \
                 tc.tile_pool(name="pa_st", bufs=3) as pst, \
                 tc.tile_pool(name="pa_ps", bufs=2, space="PSUM") as pat, \
                 tc.tile_pool(name="pa_ps2", bufs=2, space="PSUM") as pap:

                def ln_block(dsts, srcs):
                    """LayerNorm a block of NT 128-row tiles with batched stats."""
                    n = len(srcs)
                    xsbs = []
                    ssum4 = pa.tile([128, NT], F32, tag="ssum4")
                    s24 = pa.tile([128, NT], F32, tag="s24")
                    sc2 = pa.tile([128, DIM], F32, tag="sc2", bufs=1)
                    sq = pa.tile([128, DIM], F32, tag="sq", bufs=1)
                    for r in range(n):
                        xsb = pa.tile([128, DIM], F32, tag="xsb", bufs=5,
                                      name=f"xsb_{r}")
                        nc.sync.dma_start(out=xsb[:, :], in_=srcs[r])
                        xsbs.append(xsb)
                        nc.scalar.activation(sc2[:, :], xsb[:, :], AF.Copy,
                                             accum_out=ssum4[:, r:r + 1])
                        nc.vector.scalar_tensor_tensor(
                            out=sq[:, :], in0=xsb[:, :], scalar=0.0,
                            in1=xsb[:, :], op0=ALU.add, op1=ALU.mult,
                            accum_out=s24[:, r:r + 1])
                    mu4 = pa.tile([128, NT], F32, tag="mu4")
                    nc.scalar.mul(mu4[:, :], ssum4[:, :], 1.0 / DIM)
                    mu24 = pa.tile([128, NT], F32, tag="mu24")
                    nc.scalar.activation(mu24[:, :], mu4[:, :], AF.Square)
                    var4 = pa.tile([128, NT], F32, tag="var4")
                    nc.vector.scalar_tensor_tensor(out=var4[:, :], in0=s24[:, :],
                                                   scalar=1.0 / DIM,
                                                   in1=mu24[:, :], op0=ALU.mult,
                                                   op1=ALU.subtract)
                    std4 = pa.tile([128, NT], F32, tag="std4")
                    nc.scalar.activation(std4[:, :], var4[:, :], AF.Sqrt,
                                         bias=eps[:, :])
                    rstd4 = pa.tile([128, NT], F32, tag="rstd4")
                    nc.vector.reciprocal(rstd4[:, :], std4[:, :])
                    for r in range(n):
                        nc.vector.tensor_scalar(out=dsts[r], in0=xsbs[r][:, :],
                                                scalar1=mu4[:, r:r + 1],
                                                op0=ALU.subtract,
                                                scalar2=rstd4[:, r:r + 1],
                                                op1=ALU.mult)
                        if has_beta:
                            nc.vector.tensor_tensor(out=dsts[r], in0=dsts[r],
                                                    in1=bet[:, :], op=ALU.add)

                def transpose6(dst_sb, src_sb, r, bf=False):
                    """6x PE transpose of src [128, 768], single batched evict."""
                    pt6 = pat.tile([128, DIM], BF16 if bf else F32,
                                   tag="pt6b" if bf else "pt6",
                                   bufs=2 if bf else 1)
                    for ct in range(6):
                        nc.tensor.transpose(pt6[:, ct * 128:(ct + 1) * 128],
                                            src_sb[:, ct * 128:(ct + 1) * 128],
                                            ident16[:, :] if bf else ident[:, :])
                    nc.scalar.copy(
                        dst_sb[:, :, r * 128:(r + 1) * 128],
                        pt6[:, :].rearrange("p (c i) -> p c i", c=6))

                def rope(dst, src, cs_sb, final_out=None):
                    """dst = src*cos + rotate_half(src)*sin via shifted APs."""
                    def rep(ap, n32):
                        return ap.rearrange("p (o d) -> p o d", o=1) \
                            .broadcast_to([128, 12, 32 * n32])
                    sv = src.rearrange("p (h d) -> p h d", h=12)
                    rot = pa.tile([128, DIM], F32, tag="rot")
                    rv = rot[:, :].rearrange("p (h d) -> p h d", h=12)
                    # rot[:, :, 0:32]  = src[:, :, 32:64] * (-sin[0:32])
                    nc.vector.tensor_tensor(out=rv[:, :, 0:32],
                                            in0=sv[:, :, 32:64],
                                            in1=rep(cs_sb[:, 128:160], 1),
                                            op=ALU.mult)
                    # rot[:, :, 32:64] = src[:, :, 0:32] * sin[32:64]
                    nc.vector.tensor_tensor(out=rv[:, :, 32:64],
                                            in0=sv[:, :, 0:32],
                                            in1=rep(cs_sb[:, 96:128], 1),
                                            op=ALU.mult)
                    dv = dst.rearrange("p (h d) -> p h d", h=12)
                    nc.vector.tensor_tensor(out=dv, in0=sv,
                                            in1=rep(cs_sb[:, 0:64], 2),
                                            op=ALU.mult)
                    nc.vector.tensor_tensor(
                        out=dst if final_out is None else final_out,
                        in0=dst, in1=rot[:, :], op=ALU.add)

                def rope16(dst, src, cs_sb, final_out=None):
                    """bf16 rope via shifted APs (DVE 4x mode)."""
                    def rep(ap, n32):
                        return ap.rearrange("p (o d) -> p o d", o=1) \
                            .broadcast_to([128, 12, 32 * n32])
                    sv = src.rearrange("p (h d) -> p h d", h=12)
                    rot = pa.tile([128, DIM], BF16, tag="rot16")
                    rv = rot[:, :].rearrange("p (h d) -> p h d", h=12)
                    nc.vector.tensor_tensor(out=rv[:, :, 0:32],
                                            in0=sv[:, :, 32:64],
                                            in1=rep(cs_sb[:, 128:160], 1),
                                            op=ALU.mult)
                    nc.vector.tensor_tensor(out=rv[:, :, 32:64],
                                            in0=sv[:, :, 0:32],
                                            in1=rep(cs_sb[:, 96:128], 1),
                                            op=ALU.mult)
                    dv = dst.rearrange("p (h d) -> p h d", h=12)
                    nc.vector.tensor_tensor(out=dv, in0=sv,
                                            in1=rep(cs_sb[:, 0:64], 2),
                                            op=ALU.mult)
                    nc.vector.tensor_tensor(
                        out=dst if final_out is None else final_out,
                        in0=dst, in1=rot[:, :], op=ALU.add)

                def project(xnT_sb, w_sb, r, dst):
                    """One 128-row natural-layout projection -> dst [128, 768]."""
                    for half in range(2):
                        pk = pap.tile([128, 384], F32, tag="pk")
                        for ct in range(6):
                            nc.tensor.matmul(
                                pk[:, :],
                                xnT_sb[:, ct, r * 128:(r + 1) * 128],
                                w_sb[:, ct, half * 384:(half + 1) * 384],
                                start=(ct == 0), stop=(ct == 5))
                        nc.scalar.copy(dst[:, half * 384:(half + 1) * 384],
                                       pk[:, :])

                # --- q block (own rows) ---
                with tc.tile_pool(name="pa_q", bufs=1) as paq:
                    wq = paq.tile([128, 6, DIM], F32R, tag="wq")
                    nc.sync.dma_start(out=wq[:, :, :],
                                      in_=wqt_in.rearrange("(t p) a -> p t a",
                                                           p=128))
                    xnTq = paq.tile([128, 6, LC], F32R, tag="xnTq", bufs=1)
                    xns = [pa.tile([128, DIM], F32, tag="xn", name=f"xnq_{r}")
                           for r in range(NT)]
                    ln_block([xn[:, :] for xn in xns],
                             [xq_in[r * 128:(r + 1) * 128, :] for r in range(NT)])
                    for r in range(NT):
                        transpose6(xnTq[:, :, :], xns[r][:, :], r)
                    for r in range(NT):
                        qn = pa.tile([128, DIM], F32, tag="qn")
                        project(xnTq, wq, r, qn[:, :])
                        csq_sb = pa.tile([128, 160], F32, tag="csq")
                        nc.sync.dma_start(out=csq_sb[:, :],
                                          in_=csq_in[r * 128:(r + 1) * 128, :])
                        qr = pa.tile([128, DIM], F32, tag="qr")
                        rope(qr[:, :], qn[:, :], csq_sb)
                        qTs = pst.tile([64, 12, 128], BF16, tag="qTs")
                        for hx in range(2):
                            pq6 = pat.tile([64, 6 * 128], F32, tag="pq6", bufs=1)
                            for hh in range(6):
                                h = hx * 6 + hh
                                nc.tensor.transpose(
                                    pq6[:, hh * 128:(hh + 1) * 128],
                                    qr[:, h * 64:(h + 1) * 64], ident[:, :])
                            nc.scalar.copy(
                                qTs[:, hx * 6:(hx + 1) * 6, :],
                                pq6[:, :].rearrange("p (c i) -> p c i", c=6))
                        nc.sync.dma_start(
                            out=qts[:, :, r * 128:(r + 1) * 128]
                            .rearrange("h p i -> p h i"),
                            in_=qTs[:, :, :])

                # --- k/v chunks ---
                with tc.tile_pool(name="pa_kv", bufs=1) as pakv:
                    wk = pakv.tile([128, 6, DIM], F32R, tag="wk")
                    nc.sync.dma_start(out=wk[:, :, :],
                                      in_=wkt_in.rearrange("(t p) a -> p t a",
                                                           p=128))
                    wv = pakv.tile([128, 6, DIM], F32R, tag="wv")
                    nc.sync.dma_start(out=wv[:, :, :],
                                      in_=wvt_in.rearrange("(t p) a -> p t a",
                                                           p=128))
                    for jc in range(NJC):
                        xnT = pst.tile([128, 6, 512], F32R, tag="xnT", bufs=2)
                        xns = [pa.tile([128, DIM], F32, tag="xn",
                                       name=f"xn_{jc}_{r}") for r in range(NT)]
                        ln_block([xn[:, :] for xn in xns],
                                 [x_in[jc * 512 + r * 128:
                                       jc * 512 + (r + 1) * 128, :]
                                  for r in range(NT)])
                        for r in range(NT):
                            transpose6(xnT[:, :, :], xns[r][:, :], r)
                        ktT = pst.tile([128, 6, 512], BF16, tag="ktT")
                        vr16 = pst.tile([128, 4, DIM], BF16, tag="vr16")
                        for r in range(NT):
                            cs_sb = pa.tile([128, 160], BF16, tag="cs")
                            nc.sync.dma_start(
                                out=cs_sb[:, :],
                                in_=cs16_in[jc * 512 + r * 128:
                                            jc * 512 + (r + 1) * 128, :])
                            kn = pa.tile([128, DIM], BF16, tag="kn")
                            project(xnT, wk, r, kn[:, :])
                            kr = pa.tile([128, DIM], BF16, tag="kr")
                            rope16(kr[:, :], kn[:, :], cs_sb)
                            transpose6(ktT[:, :, :], kr[:, :], r, bf=True)
                            vn = pa.tile([128, DIM], BF16, tag="vn")
                            project(xnT, wv, r, vn[:, :])
                            vc = pa.tile([128, DIM], BF16, tag="vc")
                            rope16(vc[:, :], vn[:, :], cs_sb,
                                   final_out=vr16[:, r, :])
                        nc.sync.dma_start(
                            out=kts[jc][:, :].rearrange("(t p) j -> p t j", p=128),
                            in_=ktT[:, :, :])
                        nc.sync.dma_start(
                            out=vs[jc][:, :].rearrange("(t p) a -> p t a", p=128),
                            in_=vr16[:, :, :])

            # ================= PHASE B =================
            with tc.tile_pool(name="pb_mask", bufs=1) as pbm, \
                 tc.tile_pool(name="pb_sb", bufs=1) as pb1, \
                 tc.tile_pool(name="pb_sm", bufs=2) as pbs, \
                 tc.tile_pool(name="pb_du", bufs=2) as pbd, \
                 tc.tile_pool(name="pb_ps", bufs=3, space="PSUM") as pbq, \
                 tc.tile_pool(name="pb_pst", bufs=3, space="PSUM") as pbt, \
                 tc.tile_pool(name="pb_pso", bufs=2, space="PSUM") as pbo:
                mask = pbm.tile([128, NT, SEQ], F32, tag="mask")
                with tc.tile_pool(name="pb_iota", bufs=1) as pbi:
                    iota = pbi.tile([128, SEQ], F32, tag="iota")
                    nc.gpsimd.iota(iota[:, :], pattern=[[1, SEQ]], base=0,
                                   channel_multiplier=0,
                                   allow_small_or_imprecise_dtypes=True)
                    for t in range(NT):
                        nc.vector.tensor_scalar(out=mask[:, t, :], in0=iota[:, :],
                                                scalar1=gidx[:, t:t + 1],
                                                scalar2=None, op0=ALU.is_gt)
                        nc.vector.tensor_scalar(out=mask[:, t, :],
                                                in0=mask[:, t, :],
                                                scalar1=float(NEG), scalar2=None,
                                                op0=ALU.mult)

                for h in range(NUM_HEADS):
                    ohs = pbs.tile([64, LC], F32R, tag="ohs")
                    qh = pb1.tile([64, LC], BF16, tag="qh", bufs=2)
                    nc.sync.dma_start(out=qh[:, :], in_=qts[h, :, :])
                    kT = pb1.tile([64, NJC, 512], BF16, tag="kT", bufs=2)
                    vh = pb1.tile([128, 32, 64], BF16, tag="vh", bufs=2)
                    for jc in range(NJC):
                        nc.sync.dma_start(out=kT[:, jc, :],
                                          in_=kts[jc][h * 64:(h + 1) * 64, :])
                        nc.sync.dma_start(
                            out=vh[:, jc * 4:(jc + 1) * 4, :],
                            in_=vs[jc][:, h * 64:(h + 1) * 64]
                            .rearrange("(t p) d -> p t d", p=128))
                    for t in range(NT):
                        du = pbd.tile([128, SEQ], F32, tag="du", bufs=3)
                        for jc in range(NJC):
                            pd = pbq.tile([128, 512], F32, tag="pd")
                            nc.tensor.matmul(pd[:, :],
                                             qh[:, t * 128:(t + 1) * 128],
                                             kT[:, jc, :], start=True, stop=True)
                            nc.vector.scalar_tensor_tensor(
                                out=du[:, jc * 512:(jc + 1) * 512], in0=pd[:, :],
                                scalar=0.0,
                                in1=mask[:, t, jc * 512:(jc + 1) * 512],
                                op0=ALU.add, op1=ALU.add)
                            if jc in (1, 3, 5):
                                lo = 1024 * (jc - 1) // 2 * 2
                                lo = {1: 0, 3: 1024, 5: 2048}[jc]
                                nc.sync.dma_start(
                                    out=dots_o[h, t * 128:(t + 1) * 128,
                                               lo:lo + 1024],
                                    in_=du[:, lo:lo + 1024])
                        nc.sync.dma_start(
                            out=dots_o[h, t * 128:(t + 1) * 128, 3072:SEQ],
                            in_=du[:, 3072:SEQ])
                        au = pbd.tile([128, SEQ], F32, tag="au")
                        rs = pbs.tile([128, 1], F32, tag="rs")
                        nc.scalar.activation(au[:, :], du[:, :], AF.Exp,
                                             accum_out=rs[:, :])
                        ri = pbs.tile([128, 1], F32, tag="ri")
                        nc.vector.reciprocal(ri[:, :], rs[:, :])
                        nc.vector.tensor_scalar(out=au[:, :], in0=au[:, :],
                                                scalar1=ri[:, :], scalar2=None,
                                                op0=ALU.mult)
                        nc.sync.dma_start(out=attn_o[h, t * 128:(t + 1) * 128, :],
                                          in_=au[:, :])
                        aT = pbs.tile([128, 32, 128], BF16, tag="aT", bufs=2)
                        for j4 in range(8):
                            pt4 = pbt.tile([128, 512], F32, tag="pt4")
                            for jj in range(4):
                                jt = j4 * 4 + jj
                                nc.tensor.transpose(
                                    pt4[:, jj * 128:(jj + 1) * 128],
                                    au[:, jt * 128:(jt + 1) * 128], ident[:, :])
                            nc.scalar.copy(
                                aT[:, j4 * 4:(j4 + 1) * 4, :],
                                pt4[:, :].rearrange("p (c i) -> p c i", c=4))
                        po = pbo.tile([64, 128], F32, tag="po")
                        for jt in range(32):
                            nc.tensor.matmul(po[:, :], vh[:, jt, :], aT[:, jt, :],
                                             start=(jt == 0), stop=(jt == 31))
                        nc.scalar.copy(ohs[:, t * 128:(t + 1) * 128], po[:, :])
                    nc.sync.dma_start(out=ots[h, :, :], in_=ohs[:, :])

            # ================= PHASE C =================
            with tc.tile_pool(name="pc_sb", bufs=2) as pc, \
                 tc.tile_pool(name="pc_w", bufs=1) as pcw, \
                 tc.tile_pool(name="pc_ps", bufs=2, space="PSUM") as pcp:
                wo = pcw.tile([64, NUM_HEADS, DIM], F32R, tag="wo")
                nc.sync.dma_start(out=wo[:, :, :],
                                  in_=wot_in.rearrange("(h d) e -> d h e", d=64))
                oT64 = pcw.tile([64, NUM_HEADS, LC], F32R, tag="oT64")
                nc.sync.dma_start(out=oT64[:, :, :],
                                  in_=ots[:, :, :].rearrange("h d i -> d h i"))
                if has_bo:
                    bo_sb = pcw.tile([128, DIM], F32, tag="bo")
                    nc.sync.dma_start(out=bo_sb[:, :], in_=bo_in[:, :])
                for t in range(NT):
                    xqb = pc.tile([128, DIM], F32, tag="xqb")
                    nc.sync.dma_start(out=xqb[:, :],
                                      in_=xq_in[t * 128:(t + 1) * 128, :])
                    if has_bo:
                        nc.vector.tensor_tensor(out=xqb[:, :], in0=xqb[:, :],
                                                in1=bo_sb[:, :], op=ALU.add)
                    osb = pc.tile([128, DIM], F32, tag="osb")
                    for half in range(2):
                        pp = pcp.tile([128, 384], F32, tag="pp")
                        for hh in range(NUM_HEADS):
                            nc.tensor.matmul(
                                pp[:, :], oT64[:, hh, t * 128:(t + 1) * 128],
                                wo[:, hh, half * 384:(half + 1) * 384],
                                start=(hh == 0), stop=(hh == NUM_HEADS - 1))
                        nc.vector.scalar_tensor_tensor(
                            out=osb[:, half * 384:(half + 1) * 384], in0=pp[:, :],
                            scalar=0.0, in1=xqb[:, half * 384:(half + 1) * 384],
                            op0=ALU.add, op1=ALU.add)
                    nc.sync.dma_start(out=out_o[t * 128:(t + 1) * 128, :],
                                      in_=osb[:, :])

    nc.compile()
    return nc


def _get_nc(has_beta=False, has_bo=False):
    key = ("nc", has_beta, has_bo)
    if key not in _CACHED:
        _CACHED[key] = _build(has_beta, has_bo)
    return _CACHED[key]


def kernel(x, Wq, Wk, Wv, Wo, bo, gamma, beta, rope):
    from concourse.bass_utils import run_bass_kernel_spmd

    x = np.asarray(x, dtype=np.float32)
    rope = np.asarray(rope, dtype=np.float32)
    x2 = x.reshape(SEQ, DIM)
    ang = rope.reshape(SEQ, HEAD_DIM)
    cosf = np.cos(ang).astype(np.float32)
    sinf = np.sin(ang).astype(np.float32)
    cs = np.concatenate([cosf, sinf, -sinf[:, 0:32]], axis=1).astype(np.float32)
    scale = np.float32(HEAD_DIM ** -0.5)
    gamma = np.asarray(gamma, np.float32)
    beta = np.asarray(beta, np.float32)
    bo = np.asarray(bo, np.float32)
    has_beta = bool(np.any(beta != 0))
    has_bo = bool(np.any(bo != 0))
    # fold gamma into the projection weights (W @ diag(gamma) transposed)
    WqT = np.ascontiguousarray((np.asarray(Wq) * scale).T.astype(np.float32)
                               * gamma[:, None])
    WkT = np.ascontiguousarray(np.asarray(Wk).T.astype(np.float32)
                               * gamma[:, None])
    WvT = np.ascontiguousarray(np.asarray(Wv).T.astype(np.float32)
                               * gamma[:, None])
    WoT = np.ascontiguousarray(np.asarray(Wo).T.astype(np.float32))
    ident = np.eye(128, dtype=np.float32)
    import ml_dtypes
    ident16 = np.eye(128, dtype=ml_dtypes.bfloat16)
    cs16 = cs.astype(ml_dtypes.bfloat16)

    in_maps = []
    for c in range(N_CORES):
        rows = slice(c * LC, (c + 1) * LC)
        gidx = (np.arange(c * LC, (c + 1) * LC, dtype=np.float32)
                .reshape(NT, 128).T.copy())
        im = {
            "x": x2, "xq": np.ascontiguousarray(x2[rows]),
            "cs": cs, "csq": np.ascontiguousarray(cs[rows]),
            "WqT": WqT, "WkT": WkT, "WvT": WvT, "WoT": WoT,
            "gidx": gidx, "ident": ident, "ident16": ident16, "cs16": cs16,
        }
        if has_beta:
            im["bet"] = np.broadcast_to(beta / np.where(gamma == 0, 1, gamma), (128, DIM)).astype(np.float32).copy()
        if has_bo:
            im["bo"] = np.broadcast_to(bo, (128, DIM)).copy()
        in_maps.append(im)

    nc = _get_nc(has_beta, has_bo)
    res = run_bass_kernel_spmd(nc, in_maps, core_ids=list(range(N_CORES)))

    out = np.empty((1, SEQ, DIM), np.float32)
    attn_map = np.empty((1, NUM_HEADS, SEQ, SEQ), np.float32)
    dots = np.empty((1, NUM_HEADS, SEQ, SEQ), np.float32)
    for c in range(N_CORES):
        r = res.results[c]
        rows = slice(c * LC, (c + 1) * LC)
        out[0, rows] = r["out"]
        attn_map[0, :, rows] = r["attn"]
        dots[0, :, rows] = r["dots"]
    return out, attn_map, dots
